# revision 2
# baseline (speedup 1.0000x reference)
"""Trainium2 Bass kernel for nn_EnergyLoss: batched 16x16 complex Hermitian
ground-state projector via shifted matrix-squaring power iteration.

Math (derived from the reference):
  H[n] = 0.5*G - 0.5*sum_d X[n,d]*S_d + (0.5*q_n + EPS)*I,
     G = sum_d A_d A_d^H,  S_d = A_d + A_d^H,  q_n = sum_d X[n,d]^2
  theta_n = tr(H^8)^(1/8) * (1+1e-4)  (host-side spectral bound >= lmax;
     much tighter than fro -> 9 squarings suffice instead of 11)
  B0 = PF*(I - H/theta)   (PSD shift, prefolded by PF=1/3)
  B <- B^2, renormalized by 1/||B||_F^2 on steps {2,4,6}   (9 steps total)
  B converges to c*P (ground-state projector); loss terms from P via rowsums.

Implementation notes:
  - complex 16x16 embedded as real 32x32 M(B) = [[Br,-Bi],[Bi,Br]]; 4 samples
    stacked per 128 partitions; per-sample squaring = one 32x32 PE-tile matmul
    with 16-col moving operand ([Br;Bi] half of M).
  - H built on PE as a 102-row contraction in split f16 precision:
    rows = (x_hi, x_lo vs W_hi, x_hi vs W_lo, const hi/lo, q hi/lo,
    theta hi/lo), so H is accurate to ~1e-6 despite f16 operands (matmul
    cost only depends on output free size, so the extra rows are free).
  - state per step: wb [128, 32*NQ] f16 holds full M; left 16 cols/quad are
    the t-form [Br;Bi] (cast directly from PSUM), right 16 cols [-Bi;Br] are
    stream-shuffled from u = t*signp.
  - theta and 1/theta are host-precomputed (polynomial in the inputs), so
    there is no on-device norm chain before the H build; a single dummy
    matmul at t~0 starts the PE p-state ramp clock during the input DMA.
  - 4-slab pipelining keeps PE (the bottleneck) continuously fed; elementwise
    work is split across Act/DVE/Pool so each stays under the PE step time.
"""

import numpy as np

N, D, DIM = 4096, 32, 16
NCORES = 8
NS = N // NCORES          # 512 samples per core
NQ = NS // 4              # 128 quads (4 samples per 128 partitions)
EPS = 1e-5
LAM = 0.1
PF = 1.0 / 3.0            # prefold of B0
KSTEPS = 9
NORM_STEPS = (2, 4, 6)
NSLAB = 4
QS = NQ // NSLAB          # 32 quads per slab
NROW = 102                # contraction rows of the H build

_prog_cache = {}

# ---- cinA packed layout (bytes per partition) -----------------------------
A_XTH = 0                 # f16 [102, 512]  1024B
A_WH = 1024               # f16 [102, 512]  1024B
A_INVF = 2048             # f32 [128, 128]  512B   (1/theta per (part, quad))
A_SIGNP = 2560            # f32 [128, 1]    4B
A_MASKB = 2564            # f32 [128, 128]  512B
CA = 3076
# ---- cinB ------------------------------------------------------------------
B_WPOS = 0                # f32 [128, 128]  512B
B_WEA2 = 512              # f32 [128, 128]  512B
B_XBLK = 1024             # f32 [128, 128]  512B
CB = 1536


def _f16(x):
    return np.asarray(x, np.float16).astype(np.float64)


def _build_host_tensors(A_real, A_imag, X):
    A = (A_real + 1j * A_imag).astype(np.complex128)
    Sc = A + np.conj(np.transpose(A, (0, 2, 1)))        # [D,16,16] Hermitian
    G = np.einsum('dij,dkj->ik', A, A.conj())
    cA = A.sum(axis=1)                                   # [D,16] colsum over i
    cA2 = (A @ A).sum(axis=1)
    Xf = np.asarray(X, np.float64)

    # per-sample spectral bound theta = tr(H^8)^(1/8) (host, f32 batched)
    Hn = (0.5 * G[None].astype(np.complex64)
          - 0.5 * np.einsum('nd,dij->nij', Xf.astype(np.float32),
                            Sc.astype(np.complex64))
          + ((0.5 * (Xf ** 2).sum(1)).astype(np.float32) + EPS)[:, None, None]
          * np.eye(DIM, dtype=np.complex64))
    H2 = Hn @ Hn
    H4 = H2 @ H2
    tr8 = np.einsum('nij,nji->n', H4, H4).real.astype(np.float64)
    theta = tr8 ** 0.125 * (1.0 + 1e-4)                  # [N]

    # ---- WH coefficient rows (t-layout col c = 32j + tr, tr<16 Re, >=16 Im)
    # pm = PF*(theta*I - H); rows: see NROW layout below.
    W = np.zeros((D, 512))                               # PF*0.5*S in t-form
    C = np.zeros(512)                                    # -PF*(0.5G + eps I)
    DIAG = np.zeros(512)                                 # delta at Re diag
    for j in range(DIM):
        c = 32 * j
        W[:, c:c + 16] = PF * 0.5 * Sc[:, :, j].real
        W[:, c + 16:c + 32] = PF * 0.5 * Sc[:, :, j].imag
        C[c:c + 16] = -PF * 0.5 * G[:, j].real
        C[c + 16:c + 32] = -PF * 0.5 * G[:, j].imag
        C[c + j] -= PF * EPS
        DIAG[c + j] = 1.0
    Whi = _f16(W)
    Wlo = W - Whi
    Chi = _f16(C)
    Clo = C - Chi
    kq = float(_f16(-PF / 2))
    kt = float(_f16(PF))

    WH = np.zeros((NROW, 512))
    WH[0:32] = Whi
    WH[32:64] = Whi
    WH[64:96] = Wlo
    WH[96] = Chi
    WH[97] = Clo
    WH[98] = kq * DIAG
    WH[99] = kq * DIAG
    WH[100] = kt * DIAG
    WH[101] = kt * DIAG

    MASKB = np.zeros((128, 128), np.float32)
    for b in range(4):
        MASKB[32 * b:32 * b + 32, 32 * b:32 * b + 32] = 1.0
    SIGNP = np.ones((128, 1), np.float32)
    for s in range(4):
        SIGNP[32 * s + 16:32 * s + 32, 0] = -1.0
    # finish functionals: rs is rowsums of t-form [Pr; +Pi]
    WPOS = np.zeros((128, 128), np.float32)
    WEA2 = np.zeros((128, 128), np.float32)
    for s in range(4):
        b = 32 * s
        WPOS[b:b + 16, b:b + 32] = cA.real.T
        WPOS[b + 16:b + 32, b:b + 32] = -cA.imag.T
        WEA2[b:b + 16, b:b + 32] = cA2.real.T
        WEA2[b + 16:b + 32, b:b + 32] = -cA2.imag.T

    def put(buf, off, arr, dt):
        a = np.ascontiguousarray(arr.astype(dt))
        b = a.view(np.uint8).reshape(a.shape[0], -1)
        buf[:a.shape[0], off:off + b.shape[1]] = b

    q_all = (Xf ** 2).sum(1)
    qv_all = q_all * (-PF / 2) / kq                      # value rows for q
    tv_all = theta * PF / kt                             # value rows for theta
    per_core = []
    for cix in range(NCORES):
        sl = slice(cix * NS, (cix + 1) * NS)
        Xc = Xf[sl]                                      # [512, 32]
        qv, tv, th = qv_all[sl], tv_all[sl], theta[sl]
        XTH = np.zeros((NROW, 512))
        XBLK = np.zeros((128, 128), np.float32)
        INVF = np.zeros((128, 128), np.float32)
        for s in range(4):
            idx = np.arange(NQ) * 4 + s                  # sample (q, s)
            xs = Xc[idx].T                               # [32, NQ]
            xhi = _f16(xs)
            XTH[0:32, 128 * s:128 * (s + 1)] = xhi
            XTH[32:64, 128 * s:128 * (s + 1)] = _f16(xs - xhi)
            XTH[64:96, 128 * s:128 * (s + 1)] = xhi
            XTH[96, 128 * s:128 * (s + 1)] = 1.0
            XTH[97, 128 * s:128 * (s + 1)] = 1.0
            qhi = _f16(qv[idx])
            XTH[98, 128 * s:128 * (s + 1)] = qhi
            XTH[99, 128 * s:128 * (s + 1)] = _f16(qv[idx] - qhi)
            thi = _f16(tv[idx])
            XTH[100, 128 * s:128 * (s + 1)] = thi
            XTH[101, 128 * s:128 * (s + 1)] = _f16(tv[idx] - thi)
            XBLK[32 * s:32 * s + 32, :] = xs.astype(np.float32)
            INVF[32 * s:32 * s + 32, :] = (1.0 / th[idx])[None, :].astype(
                np.float32)
        bufA = np.zeros((128, CA), np.uint8)
        put(bufA, A_XTH, XTH, np.float16)
        put(bufA, A_WH, WH, np.float16)
        put(bufA, A_INVF, INVF, np.float32)
        put(bufA, A_SIGNP, SIGNP, np.float32)
        put(bufA, A_MASKB, MASKB, np.float32)
        bufB = np.zeros((128, CB), np.uint8)
        put(bufB, B_WPOS, WPOS, np.float32)
        put(bufB, B_WEA2, WEA2, np.float32)
        put(bufB, B_XBLK, XBLK, np.float32)
        per_core.append({"cina": bufA, "cinb": bufB})
    return per_core


def build_program(ksteps=KSTEPS, norm_steps=NORM_STEPS, debug=False):
    import concourse.bass as bass
    import concourse.bass_isa as bass_isa
    import concourse.bacc as bacc
    import concourse.mybir as mybir
    import concourse.tile as tile
    from contextlib import ExitStack

    f16, f32 = mybir.dt.float16, mybir.dt.float32
    u8, u32 = mybir.dt.uint8, mybir.dt.uint32
    Alu = mybir.AluOpType
    Act = mybir.ActivationFunctionType
    X_AX = mybir.AxisListType.X
    HSWAP = list(range(16, 32)) + list(range(0, 16))
    norm_set = set(norm_steps)

    nc = bacc.Bacc()
    d_cina = nc.dram_tensor("cina", [128, CA], u8, kind="ExternalInput")
    d_cinb = nc.dram_tensor("cinb", [128, CB], u8, kind="ExternalInput")
    d_out = nc.dram_tensor("out", [128, NQ], f32, kind="ExternalOutput")
    if debug:
        d_dbg_wb0 = nc.dram_tensor("dbg_wb0", [128, 4096], f16,
                                   kind="ExternalOutput")
        d_dbg_wbs = {
            k: nc.dram_tensor(f"dbg_wbs{k}", [128, 4096], f16,
                              kind="ExternalOutput")
            for k in range(KSTEPS)
        }
        d_dbg_rs = nc.dram_tensor("dbg_rs", [128, 128], f32,
                                  kind="ExternalOutput")
        d_dbg_prt = nc.dram_tensor("dbg_prt", [128, 128], f32,
                                   kind="ExternalOutput")

    with tile.TileContext(nc) as tc, ExitStack() as ctx:
        cpool = ctx.enter_context(tc.tile_pool(name="consts", bufs=1))
        spool = ctx.enter_context(tc.tile_pool(name="state", bufs=3))
        wpool = ctx.enter_context(tc.tile_pool(name="work", bufs=3))
        upool = ctx.enter_context(tc.tile_pool(name="uslab", bufs=6))
        qpool = ctx.enter_context(tc.tile_pool(name="small", bufs=3))
        pmpool = ctx.enter_context(tc.tile_pool(name="psum_pm", bufs=4,
                                                space="PSUM"))
        smpool = ctx.enter_context(tc.tile_pool(name="psum_sm", bufs=2,
                                                space="PSUM"))

        # ---------------- dummy mm at t~0: starts PE p-state ramp ----------
        wz = wpool.tile([32, 48], f16, tag="wz")
        nc.vector.memset(wz[:, :], 0.0)
        pdum = smpool.tile([16, 16], f32, tag="fin")
        nc.tensor.matmul(pdum[:, :], wz[:, 0:16], wz[:, 16:32],
                         start=True, stop=True)
        # preload activation tables off the critical path
        wact = wpool.tile([32, 16], f32, tag="wact")
        nc.scalar.activation(wact[:, :], wz[:, 0:16], Act.Copy)
        nc.scalar.activation(wact[:, :], wz[:, 0:16], Act.Square)

        cina = cpool.tile([128, CA], u8, tag="cina")
        cinb = cpool.tile([128, CB], u8, tag="cinb")
        nc.sync.dma_start(cina[:, :], d_cina[:, :])
        nc.sync.dma_start(cinb[:, :], d_cinb[:, :])

        xth = cina[:, A_XTH:A_XTH + 1024].bitcast(f16)[0:NROW, :]
        wh = cina[:, A_WH:A_WH + 1024].bitcast(f16)[0:NROW, :]
        invf = cina[:, A_INVF:A_INVF + 512].bitcast(f32)
        signp = cina[:, A_SIGNP:A_SIGNP + 4].bitcast(f32)
        maskb = cina[:, A_MASKB:A_MASKB + 512].bitcast(f32)
        wpos = cinb[:, B_WPOS:B_WPOS + 512].bitcast(f32)
        wea2 = cinb[:, B_WEA2:B_WEA2 + 512].bitcast(f32)
        xblk = cinb[:, B_XBLK:B_XBLK + 512].bitcast(f32)

        # ---------------- helpers -----------------------------------------
        def wb_left(wb_t, sl):
            return wb_t[:, :].rearrange("p (q c) -> p q c", c=32)[
                :, sl * QS:(sl + 1) * QS, 0:16]

        def wb_right_u32(wb_t, sl):
            return wb_t[:, :].bitcast(u32).rearrange("p (q w) -> p q w", w=16)[
                :, sl * QS:(sl + 1) * QS, 8:16]

        def emit_trio(wbn, pm_t, sl, scl=None, last=False):
            """pm [128, 16*QS] -> wbn left (t-form), u, wbn right."""
            dst = wb_left(wbn, sl)
            src = pm_t[:, :].rearrange("p (q j) -> p q j", j=16)
            if scl is not None:
                nc.vector.tensor_tensor(
                    dst, src,
                    scl.unsqueeze(-1).broadcast_to([128, QS, 16]),
                    op=Alu.mult)
            else:
                nc.scalar.activation(dst, src, Act.Copy)
            if last:
                return
            us = upool.tile([128, 16 * QS], f16, tag=f"u{sl % 2}")
            nc.vector.tensor_scalar_mul(
                us[:, :].rearrange("p (q j) -> p q j", j=16),
                wb_left(wbn, sl), signp[:, :])
            nc.vector.stream_shuffle(
                wb_right_u32(wbn, sl),
                us[:, :].bitcast(u32).rearrange("p (q w) -> p q w", w=8),
                mask=HSWAP)

        def emit_prep_sq(pm_t, sq_t):
            """squares for fro^2, straight from the squaring PSUM (Act)."""
            nc.scalar.activation(
                sq_t[:, :].rearrange("p (q j) -> p q j", j=16),
                pm_t[:, :].rearrange("p (q j) -> p q j", j=16), Act.Square)

        def emit_prep_red(sq_t, pr_t):
            """j-reduce of the squares -> per-partition partials."""
            nc.vector.tensor_reduce(
                pr_t[:, :], sq_t[:, :].rearrange("p (q j) -> p q j", j=16),
                axis=X_AX, op=Alu.add)

        def emit_prep_trp(pr_t, sl, trp_t, inv_t):
            """per-block partition sums (PE) + reciprocal."""
            nc.tensor.matmul(trp_t[:, sl * QS:(sl + 1) * QS], maskb[:, :],
                             pr_t[:, :], start=True, stop=True)
            nc.vector.reciprocal(inv_t[:, sl * QS:(sl + 1) * QS],
                                 trp_t[:, sl * QS:(sl + 1) * QS])

        # ---------------- phase 1: H build -> B0 ---------------------------
        wb = spool.tile([128, 32 * NQ], f16, tag="wb")
        for sl in range(NSLAB):
            ph = pmpool.tile([128, 16 * QS], f32, tag="pm")
            for j in range(DIM):
                for s in range(4):
                    nc.tensor.matmul(
                        ph[32 * s:32 * s + 32, 32 * j:32 * j + 32],
                        wh[:, 32 * j:32 * j + 32],
                        xth[:, 128 * s + QS * sl:128 * s + QS * sl + QS],
                        start=True, stop=True, tile_position=(0, 32 * s))
            # cast1 with per-quad 1/theta (DVE), u on Act, shuffle DVE
            dst = wb_left(wb, sl)
            nc.vector.tensor_tensor(
                dst, ph[:, :].rearrange("p (j q) -> p q j", j=16),
                invf[:, QS * sl:QS * (sl + 1)].unsqueeze(-1)
                    .broadcast_to([128, QS, 16]),
                op=Alu.mult)
            us = upool.tile([128, 16 * QS], f16, tag=f"u{sl % 2}")
            nc.scalar.activation(us[:, :], wb_left(wb, sl), Act.Copy,
                                 scale=signp[:, :])
            nc.vector.stream_shuffle(
                wb_right_u32(wb, sl),
                us[:, :].bitcast(u32).rearrange("p (q w) -> p q w", w=8),
                mask=HSWAP)

        if debug:
            nc.sync.dma_start(d_dbg_wb0[:, :], wb[:, :])

        # ---------------- iteration ----------------------------------------
        rs = wpool.tile([128, NQ], f32, tag="rs")
        pos = smpool.tile([128, NQ], f32, tag="fin")
        ea2 = smpool.tile([128, NQ], f32, tag="fin")
        r = wpool.tile([128, NQ], f32, tag="r")
        posn = wpool.tile([128, NQ], f32, tag="posn")
        ea2n = wpool.tile([128, NQ], f32, tag="ea2n")
        terr = wpool.tile([128, NQ], f32, tag="terr")
        t2 = wpool.tile([128, NQ], f32, tag="t2")
        p2 = wpool.tile([128, NQ], f32, tag="p2")
        vv = wpool.tile([128, NQ], f32, tag="vv")
        fin_done = set()
        fin_invt = None

        def emit_finish_rowsum(wb_t, sl):
            nc.vector.tensor_reduce(
                rs[:, sl * QS:(sl + 1) * QS], wb_left(wb_t, sl), axis=X_AX,
                op=Alu.add)

        def emit_finish_slab(sl):
            if sl in fin_done:
                return
            fin_done.add(sl)
            c = slice(sl * QS, (sl + 1) * QS)
            nc.tensor.matmul(pos[:, c], wpos[:, :], rs[:, c], start=True,
                             stop=True)
            nc.tensor.matmul(ea2[:, c], wea2[:, :], rs[:, c], start=True,
                             stop=True)
            nc.vector.tensor_tensor(posn[:, c], pos[:, c], fin_invt[:, c],
                                    op=Alu.mult)
            nc.vector.tensor_tensor(ea2n[:, c], ea2[:, c], fin_invt[:, c],
                                    op=Alu.mult)
            nc.gpsimd.tensor_tensor(terr[:, c], posn[:, c], xblk[:, c],
                                    op=Alu.subtract)
            nc.scalar.activation(t2[:, c], terr[:, c], Act.Square)
            nc.scalar.activation(p2[:, c], posn[:, c], Act.Square)
            nc.gpsimd.tensor_tensor(vv[:, c], ea2n[:, c], p2[:, c],
                                    op=Alu.subtract)
            nc.vector.scalar_tensor_tensor(r[:, c], vv[:, c], LAM, t2[:, c],
                                           op0=Alu.mult, op1=Alu.add)
            if sl == 1:
                nc.sync.dma_start(d_out[:, 0:2 * QS], r[:, 0:2 * QS])
            elif sl == 3:
                nc.scalar.dma_start(d_out[:, 2 * QS:NQ], r[:, 2 * QS:NQ])

        pending = None      # (sq tiles, pr tiles, trp, inv) in-flight window
        inv_t = None
        for k in range(ksteps):
            last = (k == ksteps - 1)
            is_norm = k in norm_set
            prep_next = (k + 1) in norm_set or k == ksteps - 2
            consume = pending is not None
            if consume:
                sqs_c, prs_c, trp_c, inv_t = pending
            wbn = spool.tile([128, 32 * NQ], f16, tag="wb")
            if prep_next:
                inv_next = qpool.tile([128, NQ], f32, tag="scl")
                trp_next = smpool.tile([128, NQ], f32, tag="trp")
                sqs_next = []
                prs_next = []
                for i in range(NSLAB):
                    pr_i = qpool.tile([128, QS], f32, tag=f"pr{i}")
                    prs_next.append(pr_i)
            for sl in range(NSLAB):
                pm = pmpool.tile([128, 16 * QS], f32, tag="pm")
                for qq in range(QS):
                    q = sl * QS + qq
                    for s in range(4):
                        nc.tensor.matmul(
                            pm[32 * s:32 * s + 32, 16 * qq:16 * qq + 16],
                            wb[32 * s:32 * s + 32, 32 * q:32 * q + 32],
                            wb[32 * s:32 * s + 32, 32 * q:32 * q + 16],
                            start=True, stop=True,
                            tile_position=(32 * s, 32 * s))
                if consume:
                    if sl < NSLAB - 1:
                        # red[sl+1] one slab ahead of its trp/recip
                        emit_prep_red(sqs_c[sl + 1], prs_c[sl + 1])
                    emit_prep_trp(prs_c[sl], sl, trp_c, inv_t)
                if is_norm:
                    emit_trio(wbn, pm, sl,
                              scl=inv_t[:, sl * QS:(sl + 1) * QS], last=last)
                else:
                    emit_trio(wbn, pm, sl, last=last)
                if last:
                    fin_invt = inv_t
                    emit_finish_rowsum(wbn, sl)
                    if sl >= 2:
                        emit_finish_slab(sl - 2)
                if prep_next:
                    sq = qpool.tile([128, 16 * QS], f32, tag=f"sq{sl}")
                    emit_prep_sq(pm, sq)
                    sqs_next.append(sq)
            if prep_next:
                # red[0] of the new window at the prep tail
                emit_prep_red(sqs_next[0], prs_next[0])
                pending = (sqs_next, prs_next, trp_next, inv_next)
            else:
                pending = None
            wb = wbn
            if debug:
                nc.sync.dma_start(d_dbg_wbs[k][:, :], wb[:, :])

        # ---------------- finish (emitted interleaved with last step) ------
        for sl in range(NSLAB):
            emit_finish_slab(sl)

        if debug:
            nc.sync.dma_start(d_dbg_rs[:, :], rs[:, :])
            nc.sync.dma_start(d_dbg_prt[:, :], fin_invt[:, :])
    nc.compile()
    return nc


def kernel(A_real, A_imag, X):
    from concourse.bass_utils import run_bass_kernel_spmd

    per_core = _build_host_tensors(
        np.asarray(A_real, np.float32), np.asarray(A_imag, np.float32),
        np.asarray(X, np.float32))

    if "nc" not in _prog_cache:
        _prog_cache["nc"] = build_program()
    nc = _prog_cache["nc"]

    res = run_bass_kernel_spmd(nc, per_core, list(range(NCORES)))
    total = 0.0
    for c in range(NCORES):
        total += float(np.asarray(res.results[c]["out"], np.float64).sum())
    return np.float32(total / N)
